# revision 1
# baseline (speedup 1.0000x reference)
"""ALSHConvNet on 8 TRN2 NeuronCores — pure data parallel (batch/8 per core).

Per core (512 samples):
- Convs as fp16 banded matmuls on TensorE, fp32 PSUM accumulation. Output
  pixel chunks packed into M with channels: M-order (parity, xpair, ch) so
  maxpool-x partners are the two contiguous partition halves.
- ALSH hash/mask path in fp32 (PE reductions, DVE/ACT elementwise, rank-2
  XNOR matmul for the [ch x batch] mask map).
- Maxpool: y-pairs = DVE max of adjacent column tiles; x-pairs = SBUF->SBUF
  DMA of the odd partition half + DVE max.
- Host does layout only: sharding, conv1 im2col, banded weight matrices,
  padding, constant selectors.
"""

import sys

for p in ("/opt/trn_rl_repo",):
    if p not in sys.path:
        sys.path.insert(0, p)

import numpy as np

import concourse.bass as bass  # noqa
import concourse.bacc as bacc
import concourse.mybir as mybir
import concourse.tile as tile
from concourse import bass_isa
from concourse.bass_utils import run_bass_kernel_spmd

F32 = mybir.dt.float32
F16 = mybir.dt.float16
AF = mybir.ActivationFunctionType
ALU = mybir.AluOpType
AX = mybir.AxisListType

NCORES = 8
B = 512
R = 0.2
EPS = 1e-12

_CACHED = {}


# ---------------------------------------------------------------- host prep
def _band_lhsT1(W1):
    l0 = np.zeros((108, 128), np.float32)
    l1 = np.zeros((72, 128), np.float32)
    for par in range(2):
        for oxp in range(4):
            for co in range(16):
                m = par * 64 + oxp * 16 + co
                oxl = 2 * oxp + par
                for ky in range(5):
                    for ci in range(3):
                        for kx in range(5):
                            wx = oxl + kx
                            if ky < 3:
                                l0[ky * 36 + ci * 12 + wx, m] = W1[co, ci, ky, kx]
                            else:
                                l1[(ky - 3) * 36 + ci * 12 + wx, m] = W1[co, ci, ky, kx]
    return l0.astype(np.float16), l1.astype(np.float16)


def _band_lhsT2(W2):
    l = np.zeros((5, 128, 80), np.float32)
    for ky in range(5):
        for par in range(2):
            for oxp in range(2):
                for co in range(20):
                    m = par * 40 + oxp * 20 + co
                    oxl = 2 * oxp + par
                    for ci in range(16):
                        for kx in range(5):
                            l[ky, (oxl + kx) * 16 + ci, m] = W2[co, ci, ky, kx]
    return l.astype(np.float16)


def _band_lhsT3(W3):
    l = np.zeros((5, 120, 40), np.float32)
    for ky in range(5):
        for par in range(2):
            for co in range(20):
                m = par * 20 + co
                for ci in range(20):
                    for kx in range(5):
                        l[ky, (par + kx) * 20 + ci, m] = W3[co, ci, ky, kx]
    return l.astype(np.float16)


def _fc_lhsT(Wo):
    l = np.zeros((4, 80, 10), np.float32)
    for d in range(4):
        for oyp in range(4):
            for co in range(20):
                l[d, oyp * 20 + co, :] = Wo[:, co * 16 + oyp * 4 + d]
    return l.astype(np.float16)


def _im2col1(xs):
    xp = np.zeros((B, 3, 36, 36), np.float16)
    xp[:, :, 2:34, 2:34] = xs.astype(np.float16)
    g0 = np.empty((4, 108, 32, B), np.float16)
    g1 = np.empty((4, 72, 32, B), np.float16)
    for c in range(4):
        for ky in range(5):
            blk = xp[:, :, ky : ky + 32, 8 * c : 8 * c + 12].transpose(1, 3, 2, 0)
            if ky < 3:
                g0[c, ky * 36 : (ky + 1) * 36].reshape(3, 12, 32, B)[:] = blk
            else:
                g1[c, (ky - 3) * 36 : (ky - 2) * 36].reshape(3, 12, 32, B)[:] = blk
    return g0, g1


def _morder(nrep_par, nxp, C):
    return [co for _ in range(nrep_par) for _ in range(nxp) for co in range(C)]


def _host_prep(inputs):
    x = inputs["x"].astype(np.float32)
    l10, l11 = _band_lhsT1(inputs["W1"].astype(np.float32))
    b1 = inputs["b1"].astype(np.float32)
    b2 = inputs["b2"].astype(np.float32)
    b3 = inputs["b3"].astype(np.float32)
    shared = {
        "l1g0": l10,
        "l1g1": l11,
        "l2": _band_lhsT2(inputs["W2"].astype(np.float32)),
        "l3": _band_lhsT3(inputs["W3"].astype(np.float32)),
        "lo": _fc_lhsT(inputs["Wo"].astype(np.float32)),
        "s1": np.repeat(np.eye(3, dtype=np.float32), 32, axis=0),
        "s2": np.tile(np.eye(16, dtype=np.float16), (4, 1)),
        "s3": np.tile(np.eye(20, dtype=np.float16), (2, 1)),
        "hw1": inputs["W1"].reshape(16, 75).astype(np.float32),
        "hw2": inputs["W2"].reshape(20, 400).astype(np.float32),
        "hw3": inputs["W3"].reshape(20, 500).astype(np.float32),
        "a1r": inputs["a1"][:75].reshape(3, 25).astype(np.float32),
        "a1t": inputs["a1"][75:].reshape(1, 5).astype(np.float32),
        "a2r": inputs["a2"][:400].reshape(16, 25).astype(np.float32),
        "a2t": inputs["a2"][400:].reshape(1, 5).astype(np.float32),
        "a3r": inputs["a3"][:500].reshape(20, 25).astype(np.float32),
        "a3t": inputs["a3"][500:].reshape(1, 5).astype(np.float32),
        "c1": inputs["c1"].reshape(1, 1).astype(np.float32),
        "c2": inputs["c2"].reshape(1, 1).astype(np.float32),
        "c3": inputs["c3"].reshape(1, 1).astype(np.float32),
        "b1m": b1[np.array(_morder(2, 4, 16))].reshape(-1, 1),
        "b2m": b2[np.array(_morder(2, 2, 20))].reshape(-1, 1),
        "b3m": b3[np.array(_morder(2, 1, 20))].reshape(-1, 1),
        "bo": inputs["bo"].reshape(10, 1).astype(np.float32),
        "ones_row": np.ones((1, 20), np.float32),
        "ones_col": np.ones((20, 1), np.float32),
    }
    in_maps = []
    for i in range(NCORES):
        xs = x[i * B : (i + 1) * B]
        g0, g1 = _im2col1(xs)
        m = dict(shared)
        m["rhs1g0"] = g0
        m["rhs1g1"] = g1
        m["xq"] = np.ascontiguousarray(xs.transpose(1, 2, 0, 3).reshape(96, B * 32))
        in_maps.append(m)
    return in_maps


# ---------------------------------------------------------------- device build
def _parity_ge1(nc, pool, t_ap, C, outtile):
    """outtile = (floor(t) mod 2) as 0/1 via fp32 magic rounding.
    Valid because |t| < 2^21 and t is >=5e-5 away from every integer."""
    MAGIC = 12582912.0  # 1.5 * 2^23
    a = pool.tile([C, t_ap.shape[1]], F32, tag="par_a", name="par_a")
    nc.vector.tensor_scalar(a[:], t_ap, 0.5, -0.5, ALU.mult, ALU.add)
    nc.vector.tensor_scalar_add(a[:], a[:], MAGIC)
    nc.vector.tensor_scalar_add(a[:], a[:], -MAGIC)  # a = floor(t/2)
    u = pool.tile([C, t_ap.shape[1]], F32, tag="par_u", name="par_u")
    nc.vector.scalar_tensor_tensor(u[:], a[:], -2.0, t_ap, ALU.mult, ALU.add)
    nc.vector.tensor_scalar(outtile, u[:], 1.0, None, ALU.is_ge)


def _bcast_row(nc, pool, pspool, row_ap, C, ones_row, tag, dtype=F32):
    """[C, N] tile = broadcast of row_ap [1, N] to C partitions (rank-1 PE)."""
    N = row_ap.shape[1]
    ps = pspool.tile([C, N], F32, tag="accps", name="bc_ps")
    nc.tensor.matmul(ps[:], ones_row[0:1, 0:C], row_ap, start=True, stop=True)
    t = pool.tile([C, N], dtype, tag=tag, name=tag)
    nc.vector.tensor_copy(t[:], ps[:])
    return t


def _hash_bits(nc, pool, pspool, Kf, aflat, atail, cc, C, ones_row):
    """kh [C,1] fp32 from weight matrix tile Kf [C, D]."""
    D = Kf.shape[1]
    sq = pool.tile([C, D], F32, tag="hsq", name="hsq")
    nc.vector.tensor_tensor(sq[:], Kf[:], Kf[:], ALU.mult)
    n2 = pool.tile([C, 1], F32, tag="hn2", name="hn2")
    nc.vector.tensor_reduce(n2[:], sq[:], AX.X, ALU.add)
    nrm = pool.tile([C, 1], F32, tag="hnrm", name="hnrm")
    nc.scalar.activation(nrm[:], n2[:], AF.Sqrt)
    nrow = pool.tile([1, C], F32, tag="hnrow", name="hnrow")
    nc.sync.dma_start(nrow[0:1, :], nrm[:, 0:1])
    nmx = pool.tile([1, 1], F32, tag="hnmx", name="hnmx")
    nc.vector.tensor_reduce(nmx[:], nrow[:], AX.X, ALU.max)
    nc.vector.tensor_scalar_add(nmx[:], nmx[:], EPS)
    rm = pool.tile([1, 1], F32, tag="hrm", name="hrm")
    nc.vector.reciprocal(rm[:], nmx[:])
    rmax = _bcast_row(nc, pool, pspool, rm[0:1, 0:1], C, ones_row, "hrmax")
    n = pool.tile([C, 1], F32, tag="hn", name="hn")
    nc.vector.tensor_tensor(n[:], nrm[:], rmax[:, 0:1], ALU.mult)
    pw = pool.tile([C, 5], F32, tag="hpw", name="hpw")
    nc.vector.tensor_tensor(pw[:, 0:1], n[:], n[:], ALU.mult)
    for i in range(1, 5):
        nc.vector.tensor_tensor(
            pw[:, i : i + 1], pw[:, i - 1 : i], pw[:, i - 1 : i], ALU.mult
        )
    atb = _bcast_row(nc, pool, pspool, atail[0:1, :], C, ones_row, "hatb")
    nc.vector.tensor_tensor(pw[:], pw[:], atb[:], ALU.mult)
    sb = pool.tile([C, 1], F32, tag="hsb", name="hsb")
    nc.vector.tensor_reduce(sb[:], pw[:], AX.X, ALU.add)
    kn = pool.tile([C, D], F32, tag="hkn", name="hkn")
    nc.vector.tensor_scalar(kn[:], Kf[:], rmax[:, 0:1], None, ALU.mult)
    arb = _bcast_row(nc, pool, pspool, aflat[0:1, :], C, ones_row, "harb")
    nc.vector.tensor_tensor(kn[:], kn[:], arb[:], ALU.mult)
    sa = pool.tile([C, 1], F32, tag="hsa", name="hsa")
    nc.vector.tensor_reduce(sa[:], kn[:], AX.X, ALU.add)
    kv = pool.tile([C, 1], F32, tag="hkv", name="hkv")
    nc.vector.tensor_tensor(kv[:], sa[:], sb[:], ALU.add)
    ccb = _bcast_row(nc, pool, pspool, cc[0:1, 0:1], C, ones_row, "hccb")
    nc.vector.tensor_tensor(kv[:], kv[:], ccb[:, 0:1], ALU.add)
    nc.vector.tensor_scalar_mul(kv[:], kv[:], 1.0 / R)
    kh = pool.tile([C, 1], F32, tag="hkh", name="hkh")
    _parity_ge1(nc, pool, kv[:], C, kh[:])
    return kh


def _query_bits(nc, pool, pspool, cms, Av, tail, cc, C, ones_col, tag):
    num_ps = pspool.tile([1, B], F32, tag="accps", name="qnum_ps")
    nc.tensor.matmul(num_ps[:], Av[:, 0:1], cms[:], start=True, stop=True)
    sqt = pool.tile([C, B], F32, tag="q_t", name="q_t")
    nc.vector.tensor_tensor(sqt[:], cms[:], cms[:], ALU.mult)
    s2_ps = pspool.tile([1, B], F32, tag="accps", name="qs2_ps")
    nc.tensor.matmul(s2_ps[:], ones_col[0:C, 0:1], sqt[:], start=True, stop=True)
    den = pool.tile([1, B], F32, tag="q_den", name="q_den")
    nc.scalar.activation(den[0:1, :], s2_ps[0:1, :], AF.Sqrt)
    nc.vector.tensor_scalar_mul(den[0:1, :], den[0:1, :], 5.0)
    nc.vector.tensor_scalar_add(den[0:1, :], den[0:1, :], EPS)
    rden = pool.tile([1, B], F32, tag="q_rden", name="q_rden")
    nc.vector.reciprocal(rden[0:1, :], den[0:1, :])
    qh = pool.tile([1, B], F32, tag=tag, name=tag)
    nc.vector.tensor_tensor(qh[0:1, :], num_ps[0:1, :], rden[0:1, :], ALU.mult)
    nc.vector.tensor_scalar_add(qh[0:1, :], qh[0:1, :], tail[0:1, 0:1])
    nc.vector.tensor_scalar_add(qh[0:1, :], qh[0:1, :], cc[0:1, 0:1])
    nc.vector.tensor_scalar_mul(qh[0:1, :], qh[0:1, :], 1.0 / R)
    _parity_ge1(nc, pool, qh[0:1, :], 1, qh[0:1, :])
    return qh


def _mask_map(nc, pool, pspool, kh, qh, P, tag):
    """[P, B] fp16 = XNOR(kh[ch(p)], qh[b]); p-order cycles channels fastest."""
    C = kh.shape[0]
    khrow = pool.tile([1, 2 * C], F32, tag="mmkhrow", name="mmkhrow")
    nc.sync.dma_start(khrow[0:1, 0:C], kh[:, 0:1])
    nc.vector.tensor_scalar(
        khrow[0:1, C : 2 * C], khrow[0:1, 0:C], -1.0, 1.0, ALU.mult, ALU.add
    )
    lhsT = pool.tile([2, P], F32, tag="mmlhsT", name="mmlhsT")
    for r in range(P // C):
        nc.sync.dma_start(lhsT[0:1, r * C : (r + 1) * C], khrow[0:1, 0:C])
        nc.sync.dma_start(lhsT[1:2, r * C : (r + 1) * C], khrow[0:1, C : 2 * C])
    qrow = pool.tile([1, 2 * B], F32, tag="mmqrow", name="mmqrow")
    nc.vector.tensor_copy(qrow[0:1, 0:B], qh[0:1, :])
    nc.vector.tensor_scalar(
        qrow[0:1, B : 2 * B], qh[0:1, :], -1.0, 1.0, ALU.mult, ALU.add
    )
    rhs = pool.tile([2, B], F32, tag="mmrhs", name="mmrhs")
    nc.sync.dma_start(rhs[0:1, :], qrow[0:1, 0:B])
    nc.sync.dma_start(rhs[1:2, :], qrow[0:1, B : 2 * B])
    mm_ps = pspool.tile([P, B], F32, tag="cps", name="mmps")
    nc.tensor.matmul(mm_ps[:], lhsT[:], rhs[:], start=True, stop=True)
    mm = pool.tile([P, B], F16, tag=tag, name=tag)
    nc.vector.tensor_copy(mm[:], mm_ps[:])
    return mm


def build_kernel():
    nc = bacc.Bacc(None, target_bir_lowering=False, debug=False)

    def din(name, shape, dtype=F32):
        return nc.dram_tensor(name, list(shape), dtype, kind="ExternalInput").ap()

    rhs1g0 = din("rhs1g0", (4, 108, 32, B), F16)
    rhs1g1 = din("rhs1g1", (4, 72, 32, B), F16)
    xq = din("xq", (96, B * 32))
    l1g0 = din("l1g0", (108, 128), F16)
    l1g1 = din("l1g1", (72, 128), F16)
    l2 = din("l2", (5, 128, 80), F16)
    l3 = din("l3", (5, 120, 40), F16)
    lo = din("lo", (4, 80, 10), F16)
    s1 = din("s1", (96, 3))
    s2 = din("s2", (64, 16), F16)
    s3 = din("s3", (40, 20), F16)
    hw1 = din("hw1", (16, 75))
    hw2 = din("hw2", (20, 400))
    hw3 = din("hw3", (20, 500))
    a1r = din("a1r", (3, 25))
    a1t = din("a1t", (1, 5))
    a2r = din("a2r", (16, 25))
    a2t = din("a2t", (1, 5))
    a3r = din("a3r", (20, 25))
    a3t = din("a3t", (1, 5))
    c1 = din("c1", (1, 1))
    c2 = din("c2", (1, 1))
    c3 = din("c3", (1, 1))
    b1m = din("b1m", (128, 1))
    b2m = din("b2m", (80, 1))
    b3m = din("b3m", (40, 1))
    bo = din("bo", (10, 1))
    ones_row = din("ones_row", (1, 20))
    ones_col = din("ones_col", (20, 1))
    out = nc.dram_tensor("out", [B, 10], F32, kind="ExternalOutput").ap()

    with tile.TileContext(nc) as tc:
        with (
            tc.tile_pool(name="const", bufs=1) as cpool,
            tc.tile_pool(name="stage", bufs=2) as spool,
            tc.tile_pool(name="big", bufs=1) as bpool,
            tc.tile_pool(name="work", bufs=2) as wpool,
            tc.tile_pool(name="hash", bufs=1) as hpool,
            tc.tile_pool(name="feat", bufs=1) as fpool,
            tc.tile_pool(name="psum", bufs=6, space="PSUM") as pspool,
            tc.tile_pool(name="psacc", bufs=2, space="PSUM") as pacc,
        ):
            def load_const(ap, dtype, tag):
                t = cpool.tile(list(ap.shape), dtype, tag=tag, name=tag)
                nc.sync.dma_start(t[:], ap[:])
                return t

            L10 = load_const(l1g0, F16, "l10")
            L11 = load_const(l1g1, F16, "l11")
            L2 = [load_const(l2[k], F16, f"l2_{k}") for k in range(5)]
            L3 = [load_const(l3[k], F16, f"l3_{k}") for k in range(5)]
            LO = [load_const(lo[k], F16, f"lo_{k}") for k in range(4)]
            S1 = load_const(s1, F32, "s1")
            S2 = load_const(s2, F16, "s2")
            S3 = load_const(s3, F16, "s3")
            B1 = load_const(b1m, F32, "b1")
            B2 = load_const(b2m, F32, "b2")
            B3 = load_const(b3m, F32, "b3")
            BO = load_const(bo, F32, "bo")
            ONR = load_const(ones_row, F32, "onr")
            ONC = load_const(ones_col, F32, "onc")
            C1 = load_const(c1, F32, "c1")
            C2 = load_const(c2, F32, "c2")
            C3 = load_const(c3, F32, "c3")

            def asum(ar_ap, C, tag):
                t = hpool.tile([C, 25], F32, tag=tag + "_in", name=tag + "_in")
                nc.sync.dma_start(t[:], ar_ap[:])
                o = hpool.tile([C, 1], F32, tag=tag, name=tag)
                nc.vector.tensor_reduce(o[:], t[:], AX.X, ALU.add)
                return o

            A1v = asum(a1r, 3, "a1v")
            A2v = asum(a2r, 16, "a2v")
            A3v = asum(a3r, 20, "a3v")

            def tailsum(at_ap, tag):
                t = hpool.tile([1, 5], F32, tag=tag + "_in", name=tag + "_in")
                nc.sync.dma_start(t[:], at_ap[:])
                o = hpool.tile([1, 1], F32, tag=tag, name=tag)
                nc.vector.tensor_reduce(o[:], t[:], AX.X, ALU.add)
                nc.vector.tensor_scalar_mul(o[:], o[:], 0.5)
                return o

            T1 = tailsum(a1t, "t1")
            T2 = tailsum(a2t, "t2")
            T3 = tailsum(a3t, "t3")

            def atailraw(at_ap, tag):
                t = hpool.tile([1, 5], F32, tag=tag, name=tag)
                nc.sync.dma_start(t[:], at_ap[:])
                return t

            A1T = atailraw(a1t, "a1traw")
            A2T = atailraw(a2t, "a2traw")
            A3T = atailraw(a3t, "a3traw")

            def aflat(ar_ap, C, tag):
                t = hpool.tile([1, C * 25], F32, tag=tag, name=tag)
                for r in range(C):
                    nc.sync.dma_start(t[0:1, r * 25 : (r + 1) * 25], ar_ap[r : r + 1, :])
                return t

            A1f = aflat(a1r, 3, "a1f")
            A2f = aflat(a2r, 16, "a2f")
            A3f = aflat(a3r, 20, "a3f")

            HW1 = load_const(hw1, F32, "hw1")
            kh1 = _hash_bits(nc, hpool, pacc, HW1, A1f, A1T, C1, 16, ONR)
            HW2 = load_const(hw2, F32, "hw2")
            kh2 = _hash_bits(nc, hpool, pacc, HW2, A2f, A2T, C2, 20, ONR)
            HW3 = load_const(hw3, F32, "hw3")
            kh3 = _hash_bits(nc, hpool, pacc, HW3, A3f, A3T, C3, 20, ONR)

            # ---- layer-1 query hash from fp32 x (streamed in 8 col chunks)
            cm1_ps = pacc.tile([3, B], F32, tag="accps")
            NBC = 16
            bw = B // NBC  # 32 samples, 1024 cols per chunk
            for bc in range(NBC):
                xt = spool.tile([96, bw * 32], F32, tag="stg_f32")
                nc.sync.dma_start(xt[:], xq[:, bc * bw * 32 : (bc + 1) * bw * 32])
                xv = xt[:].rearrange("p (b x) -> p b x", x=32)
                for xi in range(32):
                    nc.tensor.matmul(
                        cm1_ps[:, bc * bw : (bc + 1) * bw],
                        S1[:],
                        xv[:, :, xi],
                        start=(xi == 0),
                        stop=(xi == 31),
                    )
            cm1 = hpool.tile([3, B], F32, tag="cm1")
            nc.vector.tensor_copy(cm1[:], cm1_ps[:])
            qh1 = _query_bits(nc, hpool, pacc, cm1, A1v, T1, C1, 3, ONC, "qh1")
            mm1 = _mask_map(nc, hpool, pspool, kh1, qh1, 128, "mm1")

            # ---- conv1 -> H1 canonical (4 chunks [64=(oxp,ci16), 20*B])
            H1 = [fpool.tile([64, 16 * B], F16, tag=f"h1_{c}", name=f"h1_{c}") for c in range(4)]
            for c in range(4):
                for e in range(8):  # 4 oy rows -> 2 pooled rows each
                    rg0 = spool.tile([108, 4 * B], F16, tag="stg_rg0")
                    rg1 = spool.tile([72, 4 * B], F16, tag="stg_rg1")
                    nc.sync.dma_start(
                        rg0[:].rearrange("p (y b) -> p y b", y=4),
                        rhs1g0[c, :, e * 4 : (e + 1) * 4, :],
                    )
                    nc.sync.dma_start(
                        rg1[:].rearrange("p (y b) -> p y b", y=4),
                        rhs1g1[c, :, e * 4 : (e + 1) * 4, :],
                    )
                    pp = wpool.tile([128, 2 * B], F16, tag="pp")
                    for oy2 in range(2):
                        ev = []
                        for sub in range(2):
                            oy = oy2 * 2 + sub
                            ps = pspool.tile([128, B], F32, tag="cps")
                            nc.tensor.matmul(
                                ps[:], L10[:], rg0[:, oy * B : (oy + 1) * B],
                                start=True, stop=False,
                            )
                            nc.tensor.matmul(
                                ps[:], L11[:], rg1[:, oy * B : (oy + 1) * B],
                                start=False, stop=True,
                            )
                            a = wpool.tile([128, B], F16, tag="act")
                            nc.scalar.activation(a[:], ps[:], AF.Relu, bias=B1[:])
                            am = wpool.tile([128, B], F16, tag="am")
                            nc.vector.tensor_tensor(am[:], a[:], mm1[:], ALU.mult)
                            ev.append(am)
                        nc.vector.tensor_tensor(
                            pp[:, oy2 * B : (oy2 + 1) * B], ev[0][:], ev[1][:], ALU.max
                        )
                    mv = wpool.tile([64, 2 * B], F16, tag="mv")
                    nc.sync.dma_start(mv[:], pp[64:128, :])
                    oyp0 = e * 2  # H1 col row (unpadded)
                    nc.vector.tensor_tensor(
                        H1[c][:, oyp0 * B : (oyp0 + 2) * B], pp[0:64, :], mv[:], ALU.max
                    )

            # ---- layer-2 query hash
            cm2_ps = pacc.tile([16, B], F32, tag="accps")
            first = True
            for c in range(4):
                for oy in range(16):
                    nc.tensor.matmul(
                        cm2_ps[:],
                        S2[:],
                        H1[c][:, oy * B : (oy + 1) * B],
                        start=first,
                        stop=(c == 3 and oy == 15),
                    )
                    first = False
            cm2 = hpool.tile([16, B], F32, tag="cm2")
            nc.vector.tensor_copy(cm2[:], cm2_ps[:])
            qh2 = _query_bits(nc, hpool, pacc, cm2, A2v, T2, C2, 16, ONC, "qh2")
            mm2 = _mask_map(nc, hpool, pspool, kh2, qh2, 80, "mm2")

            # ---- conv2 -> H2 (4 chunks [40=(oxp,ci20), 12*B])
            H2 = [fpool.tile([40, 8 * B], F16, tag=f"h2_{d}", name=f"h2_{d}") for d in range(4)]
            for d in range(4):
                rhs = bpool.tile([128, 20 * B], F16, tag="bigrhs")
                nc.vector.memset(rhs[:, 0 : 2 * B], 0.0)
                nc.vector.memset(rhs[:, 18 * B : 20 * B], 0.0)
                if d == 0:
                    nc.vector.memset(rhs[0:32, 2 * B : 18 * B], 0.0)
                if d == 3:
                    nc.vector.memset(rhs[96:128, 2 * B : 18 * B], 0.0)
                for cc_ in range(4):
                    px_lo = max(4 * d - 2, 4 * cc_)
                    px_hi = min(4 * d + 5, 4 * cc_ + 3)
                    if px_lo > px_hi:
                        continue
                    wx0 = px_lo - (4 * d - 2)
                    src0 = (px_lo - 4 * cc_) * 16
                    np_ = (px_hi - px_lo + 1) * 16
                    nc.sync.dma_start(
                        rhs[wx0 * 16 : wx0 * 16 + np_, 2 * B : 18 * B],
                        H1[cc_][src0 : src0 + np_, :],
                    )
                for oy2 in range(8):
                    pp = wpool.tile([80, B], F16, tag="pp")
                    ev = []
                    for sub in range(2):
                        oy = oy2 * 2 + sub
                        ps = pspool.tile([80, B], F32, tag="cps")
                        for ky in range(5):
                            nc.tensor.matmul(
                                ps[:],
                                L2[ky][:],
                                rhs[:, (oy + ky) * B : (oy + ky + 1) * B],
                                start=(ky == 0),
                                stop=(ky == 4),
                            )
                        a = wpool.tile([80, B], F16, tag="act")
                        nc.scalar.activation(a[:], ps[:], AF.Relu, bias=B2[:])
                        am = wpool.tile([80, B], F16, tag="am")
                        nc.vector.tensor_tensor(am[:], a[:], mm2[:], ALU.mult)
                        ev.append(am)
                    nc.vector.tensor_tensor(pp[:], ev[0][:], ev[1][:], ALU.max)
                    mv = wpool.tile([40, B], F16, tag="mv")
                    nc.sync.dma_start(mv[:], pp[40:80, :])
                    nc.vector.tensor_tensor(
                        H2[d][:, oy2 * B : (oy2 + 1) * B],
                        pp[0:40, :],
                        mv[:],
                        ALU.max,
                    )

            # ---- layer-3 query hash
            cm3_ps = pacc.tile([20, B], F32, tag="accps")
            first = True
            for d in range(4):
                for oy in range(8):
                    nc.tensor.matmul(
                        cm3_ps[:],
                        S3[:],
                        H2[d][:, oy * B : (oy + 1) * B],
                        start=first,
                        stop=(d == 3 and oy == 7),
                    )
                    first = False
            cm3 = hpool.tile([20, B], F32, tag="cm3")
            nc.vector.tensor_copy(cm3[:], cm3_ps[:])
            qh3 = _query_bits(nc, hpool, pacc, cm3, A3v, T3, C3, 20, ONC, "qh3")
            mm3 = _mask_map(nc, hpool, pspool, kh3, qh3, 40, "mm3")

            # ---- conv3 -> H3 (4 chunks [20, 4*B]) reusing h1 slots
            H3 = [fpool.tile([20, 4 * B], F16, tag=f"h1_{d}", name=f"h3_{d}") for d in range(4)]
            for d in range(4):
                rhs = bpool.tile([128, 12 * B], F16, tag="bigrhs")
                nc.vector.memset(rhs[:, 0 : 2 * B], 0.0)
                nc.vector.memset(rhs[:, 10 * B : 12 * B], 0.0)
                if d == 0:
                    nc.vector.memset(rhs[0:64, 2 * B : 10 * B], 0.0)
                if d == 3:
                    nc.vector.memset(rhs[64:128, 2 * B : 10 * B], 0.0)
                for cc_ in range(4):
                    px_lo = max(2 * d - 2, 2 * cc_)
                    px_hi = min(2 * d + 3, 2 * cc_ + 1)
                    if px_lo > px_hi:
                        continue
                    wx0 = px_lo - (2 * d - 2)
                    src0 = (px_lo - 2 * cc_) * 20
                    np_ = (px_hi - px_lo + 1) * 20
                    nc.sync.dma_start(
                        rhs[wx0 * 20 : wx0 * 20 + np_, 2 * B : 10 * B],
                        H2[cc_][src0 : src0 + np_, :],
                    )
                for oy2 in range(4):
                    pp = wpool.tile([40, B], F16, tag="pp")
                    ev = []
                    for sub in range(2):
                        oy = oy2 * 2 + sub
                        ps = pspool.tile([40, B], F32, tag="cps")
                        for ky in range(5):
                            nc.tensor.matmul(
                                ps[:],
                                L3[ky][:],
                                rhs[0:120, (oy + ky) * B : (oy + ky + 1) * B],
                                start=(ky == 0),
                                stop=(ky == 4),
                            )
                        a = wpool.tile([40, B], F16, tag="act")
                        nc.scalar.activation(a[:], ps[:], AF.Relu, bias=B3[:])
                        am = wpool.tile([40, B], F16, tag="am")
                        nc.vector.tensor_tensor(am[:], a[:], mm3[:], ALU.mult)
                        ev.append(am)
                    nc.vector.tensor_tensor(pp[:], ev[0][:], ev[1][:], ALU.max)
                    mv = wpool.tile([20, B], F16, tag="mv")
                    nc.sync.dma_start(mv[:], pp[20:40, :])
                    nc.vector.tensor_tensor(
                        H3[d][:, oy2 * B : (oy2 + 1) * B], pp[0:20, :], mv[:], ALU.max
                    )

            # ---- FC
            fc_ps = pacc.tile([10, B], F32, tag="accps")
            for d in range(4):
                rf = wpool.tile([80, B], F16, tag="rfc")
                for oyp in range(4):
                    nc.sync.dma_start(
                        rf[oyp * 20 : (oyp + 1) * 20, :],
                        H3[d][:, oyp * B : (oyp + 1) * B],
                    )
                nc.tensor.matmul(
                    fc_ps[:], LO[d][:], rf[:], start=(d == 0), stop=(d == 3)
                )
            ob = wpool.tile([10, B], F32, tag="outb")
            nc.vector.tensor_scalar(ob[:], fc_ps[:], BO[:], None, ALU.add)
            nc.sync.dma_start(out.rearrange("b o -> o b"), ob[:])

    nc.compile()
    return nc


# ---------------------------------------------------------------- entry point
def kernel(**inputs) -> np.ndarray:
    in_maps = _host_prep(inputs)
    if "nc" not in _CACHED:
        _CACHED["nc"] = build_kernel()
    nc = _CACHED["nc"]
    res = run_bass_kernel_spmd(nc, in_maps, core_ids=list(range(NCORES)))
    outs = [res.results[i]["out"] for i in range(NCORES)]
    return np.concatenate(outs, axis=0).astype(np.float32)

